# revision 62
# baseline (speedup 1.0000x reference)
"""Causal multi-head self-attention on 8 TRN2 NeuronCores — v7 (~132us).

Sharding: batch (2) x head-group (4 heads = 256 contiguous features) -> 8
cores. Each core computes q/k/v projections for its 256 output features
from its batch's full activations, then causal attention for its 4 heads.
No collectives: the host assembles the 8 shards.

v7 design (from the 148-178us v4 baseline; all deltas trace-measured):
  - Row-tiled score pairs: heads (2h, 2h+1) live in partition halves of
    qT/kT, so their K=64 score matmuls run CONCURRENTLY on PE row-groups
    (0,0)/(64,0) via tile_position auto-derive (observed dstart ~2-4ns) —
    scores ~29us -> ~15us of PE time.
  - Batched exp: per kc, both heads' scores land in one 2-bank
    [128, 2(head), 512] PSUM tile (pool bufs=2) and ONE q0-trimmed
    activation covers them: 80 calls at the measured pace
    (172 + FD)/1.2GHz instead of 160 smaller ones. The pair tiles
    double-buffer so the next unit's scores only WAR-wait on the
    matching exp (a single 4-bank quad serialized the PE and let HAM
    re-throttle the clock: 219us!). u-pool bufs MUST stay >= 6: bufs=4
    reproducibly corrupts results (rel err 2.6e-2).
  - Causal mask applied AFTER exp as a 0/1 bf16 lower-triangle multiply
    on the DVE ([128,128] per diag block). GPSIMD is useless for this
    (TensorTensor there measured 1151ns vs 260ns on DVE, and it cannot
    access PSUM at all).
  - NO on-device softmax normalization or z transpose: PV's full-width
    [64 v | ones | 63 junk] stationary yields unnormalized z plus the
    sums row in PSUM rows 0:65; those are cast bf16 and DMA'd out as
    [65, 512] blocks; the HOST divides and transposes (also slightly
    more accurate: 5.3e-3 vs 5.6e-3).
  - v bias via a host-replicated [128, 256] tile folded into the v_aug
    fill (tensor_add), replacing 32 K=1 bias matmuls; q/k bias adds for
    the first two s-groups run on the (then idle) Act engine.
  - PE warm-up: 36 dummy matmuls on ones during the input-DMA wait so
    HAM un-throttles (4/8 -> 8/8 clock) before real work.
  - DMA: issues cost ~0.65us each on the issuing queue and all queues
    share ONE 16-engine ring (~310GB/s): the first wave (wq, x-sg0, wk)
    is ordered per-queue FIFO ahead of the 3MB of later s-groups so the
    critical set wins the ring. Outputs ship per query-group (one 3D-AP
    issue each), the final group split by head-pair.
  - NOTE: DMAs sourced from f32r-declared DRAM tensors corrupt DGE
    descriptors (found empirically); DRAM tensors are f32/bf16 only.
  - Budget at ~132us: preamble+ramp ~11, tensor-window ~112 busy /
    ~7 gaps, teardown tail ~12 (framework sem-reset storm + barrier).
"""

import sys

import ml_dtypes
import numpy as np

sys.path.insert(0, "/opt/trn_rl_repo")

import concourse.bass as bass
import concourse.tile as tile
from concourse import bacc, mybir
from concourse.bass_utils import run_bass_kernel_spmd

B, S, D, H = 2, 2048, 1024, 16
DK = D // H  # 64
NCORES = 8
HD = 256  # output features per core (4 heads x 64)
NHC = 4  # heads per core
NST = S // 128  # 16 s-tiles
NCC = D // 128  # 8 contraction chunks
NG = S // 512  # 4 query groups of 512
VW = 128  # v_aug stationary width: 64 v + 1 ones + 63 zeros (full PE width)
ZR = DK + 1  # z output rows per head: 64 z + 1 softmax-sums (host divides)

f32 = mybir.dt.float32
f32r = mybir.dt.float32r
bf16 = mybir.dt.bfloat16
AF = mybir.ActivationFunctionType
PSUM = bass.MemorySpace.PSUM


def _body(nc, tc, xt, wqt, wkt, wvt, consts, maskt, bv, out):
    with (
        tc.tile_pool(name="persist", bufs=1) as persist,
        tc.tile_pool(name="u", bufs=6) as u_pool,
        tc.tile_pool(name="zsb", bufs=2) as zsb_pool,
        tc.tile_pool(name="psum_sc", bufs=2, space=PSUM) as psum_sc,
        tc.tile_pool(name="psum_pr", bufs=2, space=PSUM) as psum_pr,
        tc.tile_pool(name="psum_z", bufs=2, space=PSUM) as psum_z,
    ):
        # ---- persistent operand tensors (all bf16) ----
        xT = persist.tile([128, NCC, S], bf16)  # 32KB/partition
        wqT = persist.tile([128, NCC, HD], bf16)
        wkT = persist.tile([128, NCC, HD], bf16)
        wvT = persist.tile([128, NCC, HD], bf16)
        # qT/kT: head-pair hdc in [128, hdc, S]; head 2*hdc in rows 0:64,
        # head 2*hdc+1 in rows 64:128 (feeds PE row-tiles directly).
        qT = persist.tile([128, 2, S], bf16)
        kT = persist.tile([128, 2, S], bf16)
        v_aug = persist.tile([128, NST, NHC, VW], bf16)
        # lower-triangle 0/1 mask (c >= p), duplicated in both hh slots so
        # one tensor_mul masks both heads' diag blocks.
        mski = persist.tile([128, 2, 128], bf16)
        cst = persist.tile([128, 4], f32)  # bq (2 cols), bk (2 cols)
        bvb = persist.tile([128, HD], bf16)  # b_v replicated across partitions

        # ---- input DMAs: 4 parallel HWDGE queues; ~0.62us issue cost
        # each, transfers ~0.3-3us via 16-engine rings. First wave feeds
        # gen_qk(0): wq on sync, x sg0 (split cc0-1 / cc2-4 / cc5-7) on
        # scalar, wk on vector, wv+consts on gpsimd.
        def pcc(ap, ncc):  # DRAM [ncc*128, w] -> iterate partition-outermost
            return ap.rearrange("(cc p) c -> p cc c", cc=ncc)

        # The critical first wave (wq | x-sg0 | wk, one queue each) must win
        # the SHARED 16-engine DMA ring; everything else queues BEHIND it on
        # the per-queue FIFOs so it can't steal ring bandwidth at startup.
        # Weights arrive as COLUMN halves matching the hdc-serial projection
        # chains' consumption order: wq_h0, wq_h1, wk_h0, wk_h1.
        nc.sync.dma_start(out=wqT[:, :, 0:128], in_=pcc(wqt[:, 0:128], NCC))
        nc.scalar.dma_start(out=xT[:, 0:2, 0:512], in_=pcc(xt[0:256, 0:512], 2))
        nc.gpsimd.dma_start(out=xT[:, 5:8, 0:512], in_=pcc(xt[640:1024, 0:512], 3))
        nc.sync.dma_start(out=wqT[:, :, 128:256], in_=pcc(wqt[:, 128:256], NCC))
        nc.scalar.dma_start(out=xT[:, 2:5, 0:512], in_=pcc(xt[256:640, 0:512], 3))
        nc.gpsimd.dma_start(out=cst[:], in_=consts)
        nc.sync.dma_start(out=wkT[:, :, 0:128], in_=pcc(wkt[:, 0:128], NCC))
        nc.gpsimd.dma_start(out=bvb[:], in_=bv[:])
        nc.gpsimd.dma_start(out=mski[:], in_=maskt)
        nc.sync.dma_start(out=wkT[:, :, 128:256], in_=pcc(wkt[:, 128:256], NCC))
        nc.scalar.dma_start(out=wvT[:], in_=pcc(wvt[:], NCC))
        nc.sync.dma_start(out=xT[:, :, 512:1024], in_=pcc(xt[:, 512:1024], NCC))
        nc.gpsimd.dma_start(out=xT[:, :, 1536:2048], in_=pcc(xt[:, 1536:2048], NCC))
        nc.sync.dma_start(out=xT[:, :, 1024:1536], in_=pcc(xt[:, 1024:1536], NCC))

        ones_row = persist.tile([1, 128], bf16)
        nc.vector.memset(ones_row[:], 1.0)
        # PE warm-up: HAM un-throttles after ~3.4us of sustained matmul
        # activity; burn the DMA wait so real matmuls start at 2.4 GHz.
        warm = psum_pr.tile([128, 128], f32, tag="pr", name="warm")
        for _ in range(36):
            nc.tensor.matmul(
                warm[:], lhsT=ones_row[:], rhs=ones_row[:], start=True, stop=True
            )
        # v_aug: zero everything once (contiguous, 4x DVE mode), then the
        # ones column; v copies later fill cols 0:64 per (st, h).
        nc.vector.memset(v_aug[:], 0.0)
        nc.vector.memset(v_aug[:, :, :, 64], 1.0)

        # ---- projections for s-group sg (yields ~0.9us sub-units) ----
        def gen_qk(sg):
            # q/k: out [hd(128) x 512] per hdc bank, accumulate over 8 ccs.
            # hdc chains run back-to-back (not interleaved) so each chain
            # only depends on its own 0.25MB weight column-half — the DMAs
            # deliver halves in consumption order during the ramp.
            for wT_t, bcol, dstT in ((wqT, 0, qT), (wkT, 2, kT)):
                pa = psum_pr.tile([128, 512], f32, tag="pr", name="pa")
                pb = psum_pr.tile([128, 512], f32, tag="pr", name="pb")
                for hdc, pp in ((0, pa), (1, pb)):
                    for cc in range(NCC):
                        nc.tensor.matmul(
                            pp[:],
                            lhsT=wT_t[:, cc, bass.ts(hdc, 128)],
                            rhs=xT[:, cc, bass.ts(sg, 512)],
                            start=(cc == 0),
                            stop=(cc == NCC - 1),
                        )
                        if cc % 4 == 3:
                            if cc == NCC - 1:
                                if sg < 2:
                                    # ramp: scalar is idle before first exps
                                    nc.scalar.add(
                                        dstT[:, hdc, bass.ts(sg, 512)],
                                        pp[:],
                                        cst[:, bcol + hdc : bcol + hdc + 1],
                                    )
                                else:
                                    nc.vector.tensor_scalar_add(
                                        dstT[:, hdc, bass.ts(sg, 512)],
                                        pp[:],
                                        cst[:, bcol + hdc : bcol + hdc + 1],
                                    )
                            yield

        def gen_v(sg, spairs=(0, 1)):
            # v: natural [s(128) x 256] per s-tile, pairs alternate banks
            for spair in spairs:
                pvs = [
                    psum_pr.tile([128, HD], f32, tag="pr", name=f"pv{i}")
                    for i in range(2)
                ]
                for cb in range(2):
                    for cc in range(4 * cb, 4 * cb + 4):
                        for stl in range(2):
                            nc.tensor.matmul(
                                pvs[stl][:],
                                lhsT=xT[:, cc, bass.ts(sg * 4 + spair * 2 + stl, 128)],
                                rhs=wvT[:, cc, :],
                                start=(cc == 0),
                                stop=(cc == NCC - 1),
                            )
                    if cb == 1:
                        for stl in range(2):
                            st = sg * 4 + spair * 2 + stl
                            nc.vector.tensor_add(
                                v_aug[:, st, :, 0:64],
                                pvs[stl][:].rearrange("p (h d) -> p h d", h=NHC),
                                bvb[:].rearrange("p (h d) -> p h d", h=NHC),
                            )
                    yield

        # ---- attention for query group g (512 queries) ----
        def gen_attn(g):
            nkc = 4 * g + 4
            # per-group output staging: unnormalized z (64 rows) + sums row
            # per head; host divides and transposes.
            zsb = zsb_pool.tile([ZR, NHC, 512], bf16, tag="zs", name="zsb")
            for hdc in (0, 1):
                zps = [
                    psum_z.tile([VW, 512], f32, tag="z", name=f"zp{hh}")
                    for hh in range(2)
                ]
                prev = None  # (kb, u_j0, u_j1)

                def flush_pv(kb, u0, u1):
                    # PV for the 4 (head, kc) units of unit kb; trim q0
                    for j, u in ((0, u0), (1, u1)):
                        kc = kb + j
                        q0 = max(0, 128 * (kc - 4 * g))
                        for hh in (0, 1):
                            nc.tensor.matmul(
                                zps[hh][:, q0:512],
                                lhsT=v_aug[:, kc, 2 * hdc + hh, :],
                                rhs=u[:, hh, q0:512],
                                start=(kc == 0),
                                stop=(kc == nkc - 1),
                            )

                for kb in range(0, nkc, 2):
                    # scores: per kc a 2-bank [128, 2(head), 512] tile; the
                    # two heads' matmuls run concurrently on PE row-groups
                    # (0,0)/(64,0). One exp per kc covers both heads
                    # (q0-trimmed strided AP). bufs=2 double-buffers so the
                    # next unit's scores only wait on the matching exp.
                    us = []
                    for j in (0, 1):
                        kc = kb + j
                        q0 = max(0, 128 * (kc - 4 * g))
                        sp = psum_sc.tile([128, 2, 512], f32, tag="sc", name="sp")
                        for hh, po in ((0, 0), (1, 64)):
                            nc.tensor.matmul(
                                sp[:, hh, q0:512],
                                lhsT=kT[po : po + 64, hdc, bass.ts(kc, 128)],
                                rhs=qT[po : po + 64, hdc, bass.ds(g * 512 + q0, 512 - q0)],
                                start=True,
                                stop=True,
                            )
                        u = u_pool.tile([128, 2, 512], bf16, tag="u", name="u")
                        nc.scalar.activation(
                            u[:, :, q0:512], sp[:, :, q0:512], AF.Exp, scale=0.125
                        )
                        # causal masking: zero upper triangle of diag block
                        if q0 > 0 or kc == 4 * g:
                            for hh in (0, 1):
                                nc.vector.tensor_mul(
                                    u[:, hh, q0 : q0 + 128],
                                    u[:, hh, q0 : q0 + 128],
                                    mski[:, 0, :],
                                )
                        us.append(u)
                    if prev is not None:
                        flush_pv(*prev)
                    prev = (kb, us[0], us[1])
                    yield
                flush_pv(*prev)

                # z tail: cast [65, 512] (64 z rows + sums row) straight to
                # bf16 SBUF; normalization and transpose happen on the host.
                # The very last pair splits across scalar (idle, exps done)
                # and vector so the tail chain is one cast deep, not two.
                for hh in (0, 1):
                    if g == NG - 1 and hdc == 1 and hh == 0:
                        nc.scalar.copy(zsb[0:ZR, 2 * hdc, :], zps[0][0:ZR, :])
                    else:
                        nc.vector.tensor_copy(
                            zsb[0:ZR, 2 * hdc + hh, :], zps[hh][0:ZR, :]
                        )
                    yield
                if g == NG - 1:
                    # tail group: ship each head-pair as soon as it's done.
                    # Issues go on gpsimd/sync — NOT scalar, whose exp stream
                    # paces the attention phases (an issue costs ~0.9us).
                    eng = (nc.gpsimd, nc.sync)[hdc]
                    eng.dma_start(
                        out=out[
                            bass.ds(hdc * 2 * ZR, 2 * ZR), bass.ts(g, 512)
                        ].rearrange("(h d) c -> d h c", h=2),
                        in_=zsb[0:ZR, 2 * hdc : 2 * hdc + 2, :],
                    )
            if g < NG - 1:
                # one issue per group (rows h*ZR+d of out) on idle gpsimd
                nc.gpsimd.dma_start(
                    out=out[:, bass.ts(g, 512)].rearrange("(h d) c -> d h c", h=NHC),
                    in_=zsb[0:ZR, :, :],
                )
            yield

        def drain(gen):
            for _ in gen:
                pass

        # program-order interleave: attention for group g alternates with the
        # projection sub-units of s-group g+1 so every engine queue mixes both
        # work streams.
        def chain2(a, b):
            yield from a
            yield from b

        drain(gen_qk(0))
        drain(gen_v(0, (0,)))
        for sg in range(NG):
            a = gen_attn(sg)
            # v projections carry no Act-engine work, so fractions of them are
            # deferred into the Act-bound later phases: half of v(0) leads the
            # attn(0) filler (so the first scores aren't queued behind all of
            # v(0) on the PE), half of v(2) into attn(2), v(3) into attn(3).
            if sg == 0:
                f = chain2(gen_v(0, (1,)), chain2(gen_qk(1), gen_v(1)))
            elif sg == 1:
                f = chain2(gen_qk(2), gen_v(2, (0,)))
            elif sg == 2:
                f = chain2(gen_v(2, (1,)), gen_qk(3))
            else:
                f = gen_v(3)
            rate = 2 if sg == NG - 1 else 1  # spread v(NG-1) across the tail
            rnd = 0
            while True:
                sa = next(a, StopIteration)
                sf = next(f, StopIteration) if rnd % rate == rate - 1 else None
                rnd += 1
                if sa is StopIteration:
                    drain(f)
                    break
                del sf


def build():
    nc = bacc.Bacc(
        "TRN2", target_bir_lowering=False, debug=False, num_devices=NCORES
    )
    xt = nc.dram_tensor("xt", [D, S], bf16, kind="ExternalInput")
    wqt = nc.dram_tensor("wqt", [D, HD], bf16, kind="ExternalInput")
    wkt = nc.dram_tensor("wkt", [D, HD], bf16, kind="ExternalInput")
    wvt = nc.dram_tensor("wvt", [D, HD], bf16, kind="ExternalInput")
    consts = nc.dram_tensor("consts", [128, 4], f32, kind="ExternalInput")
    maskt = nc.dram_tensor("maskt", [128, 2 * 128], bf16, kind="ExternalInput")
    bv = nc.dram_tensor("bv", [128, HD], bf16, kind="ExternalInput")
    out = nc.dram_tensor("out", [NHC * ZR, S], bf16, kind="ExternalOutput")
    with tile.TileContext(nc) as tc:
        _body(
            nc, tc, xt.ap(), wqt.ap(), wkt.ap(), wvt.ap(),
            consts.ap(), maskt.ap(), bv.ap(), out.ap(),
        )
    nc.compile()
    return nc


_NC_CACHE = None


def _get_nc():
    global _NC_CACHE
    if _NC_CACHE is None:
        _NC_CACHE = build()
    return _NC_CACHE


def make_in_maps(q_input, W_q, b_q, W_k, b_k, W_v, b_v):
    bf = ml_dtypes.bfloat16
    # masks+ident packed: [:, 0:128] lower-triangle (c >= p), [:, 128:256] identity
    ii = np.arange(128)
    maskt = np.zeros((128, 2 * 128), np.float32)
    maskt[:, 0:128] = (ii[None, :] >= ii[:, None]).astype(np.float32)
    maskt[:, 128:256] = maskt[:, 0:128]
    maskt = maskt.astype(bf)
    # host-side marshaling: bf16 cast + transpose (kernel-internal layout)
    xts = [np.ascontiguousarray(np.asarray(q_input[b]).T.astype(bf)) for b in range(B)]
    in_maps = []
    for c in range(NCORES):
        b = c // 4
        hs = slice((c % 4) * HD, (c % 4 + 1) * HD)
        consts = np.zeros((128, 4), np.float32)
        consts[:, 0:2] = np.asarray(b_q[hs], dtype=np.float32).reshape(2, 128).T
        consts[:, 2:4] = np.asarray(b_k[hs], dtype=np.float32).reshape(2, 128).T
        in_maps.append(
            {
                "xt": xts[b],
                "wqt": np.ascontiguousarray(np.asarray(W_q[hs]).T.astype(bf)),
                "wkt": np.ascontiguousarray(np.asarray(W_k[hs]).T.astype(bf)),
                "wvt": np.ascontiguousarray(np.asarray(W_v[hs]).T.astype(bf)),
                "consts": consts,
                "maskt": maskt,
                "bv": np.ascontiguousarray(
                    np.broadcast_to(
                        np.asarray(b_v[hs]).astype(bf).reshape(1, HD), (128, HD)
                    )
                ),
            }
        )
    return in_maps


def assemble(results):
    full = np.empty((B, S, D), dtype=np.float32)
    for c in range(NCORES):
        b = c // 4
        raw = np.asarray(results[c]["out"]).astype(np.float32)  # [NHC*ZR, S]
        zhds = raw.reshape(NHC, ZR, S)  # per head: 64 z rows + sums row
        zn = zhds[:, 0:DK, :] / zhds[:, DK : DK + 1, :]  # normalize
        h0 = (c % 4) * HD
        for h in range(NHC):
            full[b, :, h0 + h * DK : h0 + (h + 1) * DK] = zn[h].T
    return full


def _ensure_ntff_hook():
    """Register the axon NTFF profiling hook if the image's antenv lacks it."""
    try:
        from antenv import axon_hooks  # noqa: F401

        return
    except ImportError:
        pass
    import types

    try:
        from trn_agent_boot.trn_boot import _ntff_profile_via_ctypes

        hook = _ntff_profile_via_ctypes("/opt/axon/libaxon_pjrt.so")
    except Exception:
        hook = None
    mod = types.ModuleType("antenv.axon_hooks")
    mod._hook = hook
    mod.get_axon_ntff_profile_hook = lambda: mod._hook

    def _set(h):
        mod._hook = h

    mod.set_axon_ntff_profile_hook = _set
    sys.modules["antenv.axon_hooks"] = mod
    try:
        import antenv

        antenv.axon_hooks = mod
    except ImportError:
        pass


def run(inputs_dict, trace=False):
    """Run on hardware; returns (full_output, BassKernelResults)."""
    nc = _get_nc()
    if trace:
        _ensure_ntff_hook()
        import concourse.bass_utils as _bu

        _bu.upload_artifacts = lambda d: d  # no bucket access in this env
    in_maps = make_in_maps(**{k: np.asarray(v) for k, v in inputs_dict.items()})
    res = run_bass_kernel_spmd(nc, in_maps, core_ids=list(range(NCORES)), trace=trace)
    return assemble(res.results), res


def kernel(**inputs):
    out, _ = run(inputs, trace=False)
    return out


# revision 63
# speedup vs baseline: 1.1483x; 1.1483x over previous
"""Causal multi-head self-attention on 8 TRN2 NeuronCores — v7 (~132us).

Sharding: batch (2) x head-group (4 heads = 256 contiguous features) -> 8
cores. Each core computes q/k/v projections for its 256 output features
from its batch's full activations, then causal attention for its 4 heads.
No collectives: the host assembles the 8 shards.

v7 design (from the 148-178us v4 baseline; all deltas trace-measured):
  - Row-tiled score pairs: heads (2h, 2h+1) live in partition halves of
    qT/kT, so their K=64 score matmuls run CONCURRENTLY on PE row-groups
    (0,0)/(64,0) via tile_position auto-derive (observed dstart ~2-4ns) —
    scores ~29us -> ~15us of PE time.
  - Batched exp: per kc, both heads' scores land in one 2-bank
    [128, 2(head), 512] PSUM tile (pool bufs=2) and ONE q0-trimmed
    activation covers them: 80 calls at the measured pace
    (172 + FD)/1.2GHz instead of 160 smaller ones. The pair tiles
    double-buffer so the next unit's scores only WAR-wait on the
    matching exp (a single 4-bank quad serialized the PE and let HAM
    re-throttle the clock: 219us!). u-pool bufs MUST stay >= 6: bufs=4
    reproducibly corrupts results (rel err 2.6e-2).
  - Causal mask applied AFTER exp as a 0/1 bf16 lower-triangle multiply
    on the DVE ([128,128] per diag block). GPSIMD is useless for this
    (TensorTensor there measured 1151ns vs 260ns on DVE, and it cannot
    access PSUM at all).
  - NO on-device softmax normalization or z transpose: PV's full-width
    [64 v | ones | 63 junk] stationary yields unnormalized z plus the
    sums row in PSUM rows 0:65; those are cast bf16 and DMA'd out as
    [65, 512] blocks; the HOST divides and transposes (also slightly
    more accurate: 5.3e-3 vs 5.6e-3).
  - v bias via a host-replicated [128, 256] tile folded into the v_aug
    fill (tensor_add), replacing 32 K=1 bias matmuls; q/k bias adds for
    the first two s-groups run on the (then idle) Act engine.
  - PE warm-up: 36 dummy matmuls on ones during the input-DMA wait so
    HAM un-throttles (4/8 -> 8/8 clock) before real work.
  - DMA: issues cost ~0.65us each on the issuing queue and all queues
    share ONE 16-engine ring (~310GB/s): the first wave (wq, x-sg0, wk)
    is ordered per-queue FIFO ahead of the 3MB of later s-groups so the
    critical set wins the ring. Outputs ship per query-group (one 3D-AP
    issue each), the final group split by head-pair.
  - NOTE: DMAs sourced from f32r-declared DRAM tensors corrupt DGE
    descriptors (found empirically); DRAM tensors are f32/bf16 only.
  - Budget at ~132us: preamble+ramp ~11, tensor-window ~112 busy /
    ~7 gaps, teardown tail ~12 (framework sem-reset storm + barrier).
"""

import sys

import ml_dtypes
import numpy as np

sys.path.insert(0, "/opt/trn_rl_repo")

import concourse.bass as bass
import concourse.tile as tile
from concourse import bacc, mybir
from concourse.bass_utils import run_bass_kernel_spmd

B, S, D, H = 2, 2048, 1024, 16
DK = D // H  # 64
NCORES = 8
HD = 256  # output features per core (4 heads x 64)
NHC = 4  # heads per core
NST = S // 128  # 16 s-tiles
NCC = D // 128  # 8 contraction chunks
NG = S // 512  # 4 query groups of 512
VW = 128  # v_aug stationary width: 64 v + 1 ones + 63 zeros (full PE width)
ZR = DK + 1  # z output rows per head: 64 z + 1 softmax-sums (host divides)

f32 = mybir.dt.float32
f32r = mybir.dt.float32r
bf16 = mybir.dt.bfloat16
AF = mybir.ActivationFunctionType
PSUM = bass.MemorySpace.PSUM


def _body(nc, tc, xt, wqt, wkt, wvt, consts, maskt, bv, out):
    with (
        tc.tile_pool(name="persist", bufs=1) as persist,
        tc.tile_pool(name="u", bufs=6) as u_pool,
        tc.tile_pool(name="zsb", bufs=2) as zsb_pool,
        tc.tile_pool(name="psum_sc", bufs=2, space=PSUM) as psum_sc,
        tc.tile_pool(name="psum_pr", bufs=2, space=PSUM) as psum_pr,
        tc.tile_pool(name="psum_z", bufs=2, space=PSUM) as psum_z,
    ):
        # ---- persistent operand tensors (all bf16) ----
        xT = persist.tile([128, NCC, S], bf16)  # 32KB/partition
        wqT = persist.tile([128, NCC, HD], bf16)
        wkT = persist.tile([128, NCC, HD], bf16)
        wvT = persist.tile([128, NCC, HD], bf16)
        # qT/kT: head-pair hdc in [128, hdc, S]; head 2*hdc in rows 0:64,
        # head 2*hdc+1 in rows 64:128 (feeds PE row-tiles directly).
        qT = persist.tile([128, 2, S], bf16)
        kT = persist.tile([128, 2, S], bf16)
        v_aug = persist.tile([128, NST, NHC, VW], bf16)
        # lower-triangle 0/1 mask (c >= p), duplicated in both hh slots so
        # one tensor_mul masks both heads' diag blocks.
        mski = persist.tile([128, 2, 128], bf16)
        cst = persist.tile([128, 4], f32)  # bq (2 cols), bk (2 cols)
        bvb = persist.tile([128, HD], bf16)  # b_v replicated across partitions

        # ---- input DMAs: 4 parallel HWDGE queues; ~0.62us issue cost
        # each, transfers ~0.3-3us via 16-engine rings. First wave feeds
        # gen_qk(0): wq on sync, x sg0 (split cc0-1 / cc2-4 / cc5-7) on
        # scalar, wk on vector, wv+consts on gpsimd.
        def pcc(ap, ncc):  # DRAM [ncc*128, w] -> iterate partition-outermost
            return ap.rearrange("(cc p) c -> p cc c", cc=ncc)

        # The critical first wave (wq | x-sg0 | wk, one queue each) must win
        # the SHARED 16-engine DMA ring; everything else queues BEHIND it on
        # the per-queue FIFOs so it can't steal ring bandwidth at startup.
        # Weights arrive as COLUMN halves matching the hdc-serial projection
        # chains' consumption order: wq_h0, wq_h1, wk_h0, wk_h1.
        nc.sync.dma_start(out=wqT[:, :, 0:128], in_=pcc(wqt[:, 0:128], NCC))
        nc.scalar.dma_start(out=xT[:, 0:2, 0:512], in_=pcc(xt[0:256, 0:512], 2))
        nc.gpsimd.dma_start(out=xT[:, 5:8, 0:512], in_=pcc(xt[640:1024, 0:512], 3))
        nc.sync.dma_start(out=wqT[:, :, 128:256], in_=pcc(wqt[:, 128:256], NCC))
        nc.scalar.dma_start(out=xT[:, 2:5, 0:512], in_=pcc(xt[256:640, 0:512], 3))
        nc.gpsimd.dma_start(out=cst[:], in_=consts)
        nc.sync.dma_start(out=wkT[:, :, 0:128], in_=pcc(wkt[:, 0:128], NCC))
        nc.gpsimd.dma_start(out=bvb[:], in_=bv[:])
        nc.gpsimd.dma_start(out=mski[:], in_=maskt)
        nc.sync.dma_start(out=wkT[:, :, 128:256], in_=pcc(wkt[:, 128:256], NCC))
        nc.scalar.dma_start(out=wvT[:], in_=pcc(wvt[:], NCC))
        nc.sync.dma_start(out=xT[:, :, 512:1024], in_=pcc(xt[:, 512:1024], NCC))
        nc.gpsimd.dma_start(out=xT[:, :, 1536:2048], in_=pcc(xt[:, 1536:2048], NCC))
        nc.sync.dma_start(out=xT[:, :, 1024:1536], in_=pcc(xt[:, 1024:1536], NCC))

        ones_row = persist.tile([1, 128], bf16)
        nc.vector.memset(ones_row[:], 1.0)
        # PE warm-up: HAM un-throttles after ~3.4us of sustained matmul
        # activity; burn the DMA wait so real matmuls start at 2.4 GHz.
        warm = psum_pr.tile([128, 128], f32, tag="pr", name="warm")
        for _ in range(36):
            nc.tensor.matmul(
                warm[:], lhsT=ones_row[:], rhs=ones_row[:], start=True, stop=True
            )
        # v_aug: zero everything once (contiguous, 4x DVE mode), then the
        # ones column; v copies later fill cols 0:64 per (st, h).
        nc.vector.memset(v_aug[:], 0.0)
        nc.vector.memset(v_aug[:, :, :, 64], 1.0)

        # ---- projections for s-group sg (yields ~0.9us sub-units) ----
        def gen_qk(sg):
            # q/k: out [hd(128) x 512] per hdc bank, accumulate over 8 ccs.
            # hdc chains run back-to-back (not interleaved) so each chain
            # only depends on its own 0.25MB weight column-half — the DMAs
            # deliver halves in consumption order during the ramp.
            for wT_t, bcol, dstT in ((wqT, 0, qT), (wkT, 2, kT)):
                pa = psum_pr.tile([128, 512], f32, tag="pr", name="pa")
                pb = psum_pr.tile([128, 512], f32, tag="pr", name="pb")
                for hdc, pp in ((0, pa), (1, pb)):
                    for cc in range(NCC):
                        nc.tensor.matmul(
                            pp[:],
                            lhsT=wT_t[:, cc, bass.ts(hdc, 128)],
                            rhs=xT[:, cc, bass.ts(sg, 512)],
                            start=(cc == 0),
                            stop=(cc == NCC - 1),
                        )
                        if cc % 4 == 3:
                            if cc == NCC - 1:
                                if sg < 2:
                                    # ramp: scalar is idle before first exps
                                    nc.scalar.add(
                                        dstT[:, hdc, bass.ts(sg, 512)],
                                        pp[:],
                                        cst[:, bcol + hdc : bcol + hdc + 1],
                                    )
                                else:
                                    nc.vector.tensor_scalar_add(
                                        dstT[:, hdc, bass.ts(sg, 512)],
                                        pp[:],
                                        cst[:, bcol + hdc : bcol + hdc + 1],
                                    )
                            yield

        def gen_v(sg, spairs=(0, 1)):
            # v: natural [s(128) x 256] per s-tile, pairs alternate banks
            for spair in spairs:
                pvs = [
                    psum_pr.tile([128, HD], f32, tag="pr", name=f"pv{i}")
                    for i in range(2)
                ]
                for cb in range(2):
                    for cc in range(4 * cb, 4 * cb + 4):
                        for stl in range(2):
                            nc.tensor.matmul(
                                pvs[stl][:],
                                lhsT=xT[:, cc, bass.ts(sg * 4 + spair * 2 + stl, 128)],
                                rhs=wvT[:, cc, :],
                                start=(cc == 0),
                                stop=(cc == NCC - 1),
                            )
                    if cb == 1:
                        for stl in range(2):
                            st = sg * 4 + spair * 2 + stl
                            nc.vector.tensor_add(
                                v_aug[:, st, :, 0:64],
                                pvs[stl][:].rearrange("p (h d) -> p h d", h=NHC),
                                bvb[:].rearrange("p (h d) -> p h d", h=NHC),
                            )
                    yield

        # ---- attention for query group g (512 queries) ----
        def gen_attn(g):
            nkc = 4 * g + 4
            # per-group output staging: unnormalized z (64 rows) + sums row
            # per head; host divides and transposes.
            zsb = zsb_pool.tile([ZR, NHC, 512], bf16, tag="zs", name="zsb")
            for hdc in (0, 1):
                zps = [
                    psum_z.tile([VW, 512], f32, tag="z", name=f"zp{hh}")
                    for hh in range(2)
                ]
                prev = None  # (kb, u_j0, u_j1)

                def flush_pv(kb, u0, u1):
                    # PV for the 4 (head, kc) units of unit kb; trim q0
                    for j, u in ((0, u0), (1, u1)):
                        kc = kb + j
                        q0 = max(0, 128 * (kc - 4 * g))
                        for hh in (0, 1):
                            nc.tensor.matmul(
                                zps[hh][:, q0:512],
                                lhsT=v_aug[:, kc, 2 * hdc + hh, :],
                                rhs=u[:, hh, q0:512],
                                start=(kc == 0),
                                stop=(kc == nkc - 1),
                            )

                for kb in range(0, nkc, 2):
                    # scores: per kc a 2-bank [128, 2(head), 512] tile; the
                    # two heads' matmuls run concurrently on PE row-groups
                    # (0,0)/(64,0). One exp per kc covers both heads
                    # (q0-trimmed strided AP). bufs=2 double-buffers so the
                    # next unit's scores only wait on the matching exp.
                    us = []
                    for j in (0, 1):
                        kc = kb + j
                        q0 = max(0, 128 * (kc - 4 * g))
                        sp = psum_sc.tile([128, 2, 512], f32, tag="sc", name="sp")
                        for hh, po in ((0, 0), (1, 64)):
                            nc.tensor.matmul(
                                sp[:, hh, q0:512],
                                lhsT=kT[po : po + 64, hdc, bass.ts(kc, 128)],
                                rhs=qT[po : po + 64, hdc, bass.ds(g * 512 + q0, 512 - q0)],
                                start=True,
                                stop=True,
                            )
                        u = u_pool.tile([128, 2, 512], bf16, tag="u", name="u")
                        nc.scalar.activation(
                            u[:, :, q0:512], sp[:, :, q0:512], AF.Exp, scale=0.125
                        )
                        # causal masking: zero upper triangle of diag block
                        if q0 > 0 or kc == 4 * g:
                            for hh in (0, 1):
                                nc.vector.tensor_mul(
                                    u[:, hh, q0 : q0 + 128],
                                    u[:, hh, q0 : q0 + 128],
                                    mski[:, 0, :],
                                )
                        us.append(u)
                    if prev is not None:
                        flush_pv(*prev)
                    prev = (kb, us[0], us[1])
                    yield
                flush_pv(*prev)

                # z tail: cast [65, 512] (64 z rows + sums row) straight to
                # bf16 SBUF; normalization and transpose happen on the host.
                # (NB: scalar.copy here regresses 22us — AF.Copy triggers an
                # ACT table-set switch away from Exp; keep casts on the DVE.)
                for hh in (0, 1):
                    nc.vector.tensor_copy(
                        zsb[0:ZR, 2 * hdc + hh, :], zps[hh][0:ZR, :]
                    )
                    yield
                if g == NG - 1:
                    # tail group: ship each head-pair as soon as it's done.
                    # Issues go on gpsimd/sync — NOT scalar, whose exp stream
                    # paces the attention phases (an issue costs ~0.9us).
                    eng = (nc.gpsimd, nc.sync)[hdc]
                    eng.dma_start(
                        out=out[
                            bass.ds(hdc * 2 * ZR, 2 * ZR), bass.ts(g, 512)
                        ].rearrange("(h d) c -> d h c", h=2),
                        in_=zsb[0:ZR, 2 * hdc : 2 * hdc + 2, :],
                    )
            if g < NG - 1:
                # one issue per group (rows h*ZR+d of out) on idle gpsimd
                nc.gpsimd.dma_start(
                    out=out[:, bass.ts(g, 512)].rearrange("(h d) c -> d h c", h=NHC),
                    in_=zsb[0:ZR, :, :],
                )
            yield

        def drain(gen):
            for _ in gen:
                pass

        # program-order interleave: attention for group g alternates with the
        # projection sub-units of s-group g+1 so every engine queue mixes both
        # work streams.
        def chain2(a, b):
            yield from a
            yield from b

        drain(gen_qk(0))
        drain(gen_v(0, (0,)))
        for sg in range(NG):
            a = gen_attn(sg)
            # v projections carry no Act-engine work, so fractions of them are
            # deferred into the Act-bound later phases: half of v(0) leads the
            # attn(0) filler (so the first scores aren't queued behind all of
            # v(0) on the PE), half of v(2) into attn(2), v(3) into attn(3).
            if sg == 0:
                f = chain2(gen_v(0, (1,)), chain2(gen_qk(1), gen_v(1)))
            elif sg == 1:
                f = chain2(gen_qk(2), gen_v(2, (0,)))
            elif sg == 2:
                f = chain2(gen_v(2, (1,)), gen_qk(3))
            else:
                f = gen_v(3)
            rate = 2 if sg == NG - 1 else 1  # spread v(NG-1) across the tail
            rnd = 0
            while True:
                sa = next(a, StopIteration)
                sf = next(f, StopIteration) if rnd % rate == rate - 1 else None
                rnd += 1
                if sa is StopIteration:
                    drain(f)
                    break
                del sf


def build():
    nc = bacc.Bacc(
        "TRN2", target_bir_lowering=False, debug=False, num_devices=NCORES
    )
    xt = nc.dram_tensor("xt", [D, S], bf16, kind="ExternalInput")
    wqt = nc.dram_tensor("wqt", [D, HD], bf16, kind="ExternalInput")
    wkt = nc.dram_tensor("wkt", [D, HD], bf16, kind="ExternalInput")
    wvt = nc.dram_tensor("wvt", [D, HD], bf16, kind="ExternalInput")
    consts = nc.dram_tensor("consts", [128, 4], f32, kind="ExternalInput")
    maskt = nc.dram_tensor("maskt", [128, 2 * 128], bf16, kind="ExternalInput")
    bv = nc.dram_tensor("bv", [128, HD], bf16, kind="ExternalInput")
    out = nc.dram_tensor("out", [NHC * ZR, S], bf16, kind="ExternalOutput")
    with tile.TileContext(nc) as tc:
        _body(
            nc, tc, xt.ap(), wqt.ap(), wkt.ap(), wvt.ap(),
            consts.ap(), maskt.ap(), bv.ap(), out.ap(),
        )
    nc.compile()
    return nc


_NC_CACHE = None


def _get_nc():
    global _NC_CACHE
    if _NC_CACHE is None:
        _NC_CACHE = build()
    return _NC_CACHE


def make_in_maps(q_input, W_q, b_q, W_k, b_k, W_v, b_v):
    bf = ml_dtypes.bfloat16
    # masks+ident packed: [:, 0:128] lower-triangle (c >= p), [:, 128:256] identity
    ii = np.arange(128)
    maskt = np.zeros((128, 2 * 128), np.float32)
    maskt[:, 0:128] = (ii[None, :] >= ii[:, None]).astype(np.float32)
    maskt[:, 128:256] = maskt[:, 0:128]
    maskt = maskt.astype(bf)
    # host-side marshaling: bf16 cast + transpose (kernel-internal layout)
    xts = [np.ascontiguousarray(np.asarray(q_input[b]).T.astype(bf)) for b in range(B)]
    in_maps = []
    for c in range(NCORES):
        b = c // 4
        hs = slice((c % 4) * HD, (c % 4 + 1) * HD)
        consts = np.zeros((128, 4), np.float32)
        consts[:, 0:2] = np.asarray(b_q[hs], dtype=np.float32).reshape(2, 128).T
        consts[:, 2:4] = np.asarray(b_k[hs], dtype=np.float32).reshape(2, 128).T
        in_maps.append(
            {
                "xt": xts[b],
                "wqt": np.ascontiguousarray(np.asarray(W_q[hs]).T.astype(bf)),
                "wkt": np.ascontiguousarray(np.asarray(W_k[hs]).T.astype(bf)),
                "wvt": np.ascontiguousarray(np.asarray(W_v[hs]).T.astype(bf)),
                "consts": consts,
                "maskt": maskt,
                "bv": np.ascontiguousarray(
                    np.broadcast_to(
                        np.asarray(b_v[hs]).astype(bf).reshape(1, HD), (128, HD)
                    )
                ),
            }
        )
    return in_maps


def assemble(results):
    full = np.empty((B, S, D), dtype=np.float32)
    for c in range(NCORES):
        b = c // 4
        raw = np.asarray(results[c]["out"]).astype(np.float32)  # [NHC*ZR, S]
        zhds = raw.reshape(NHC, ZR, S)  # per head: 64 z rows + sums row
        zn = zhds[:, 0:DK, :] / zhds[:, DK : DK + 1, :]  # normalize
        h0 = (c % 4) * HD
        for h in range(NHC):
            full[b, :, h0 + h * DK : h0 + (h + 1) * DK] = zn[h].T
    return full


def _ensure_ntff_hook():
    """Register the axon NTFF profiling hook if the image's antenv lacks it."""
    try:
        from antenv import axon_hooks  # noqa: F401

        return
    except ImportError:
        pass
    import types

    try:
        from trn_agent_boot.trn_boot import _ntff_profile_via_ctypes

        hook = _ntff_profile_via_ctypes("/opt/axon/libaxon_pjrt.so")
    except Exception:
        hook = None
    mod = types.ModuleType("antenv.axon_hooks")
    mod._hook = hook
    mod.get_axon_ntff_profile_hook = lambda: mod._hook

    def _set(h):
        mod._hook = h

    mod.set_axon_ntff_profile_hook = _set
    sys.modules["antenv.axon_hooks"] = mod
    try:
        import antenv

        antenv.axon_hooks = mod
    except ImportError:
        pass


def run(inputs_dict, trace=False):
    """Run on hardware; returns (full_output, BassKernelResults)."""
    nc = _get_nc()
    if trace:
        _ensure_ntff_hook()
        import concourse.bass_utils as _bu

        _bu.upload_artifacts = lambda d: d  # no bucket access in this env
    in_maps = make_in_maps(**{k: np.asarray(v) for k, v in inputs_dict.items()})
    res = run_bass_kernel_spmd(nc, in_maps, core_ids=list(range(NCORES)), trace=trace)
    return assemble(res.results), res


def kernel(**inputs):
    out, _ = run(inputs, trace=False)
    return out


# revision 64
# speedup vs baseline: 1.1663x; 1.0157x over previous
"""Causal multi-head self-attention on 8 TRN2 NeuronCores — v7 (~132us).

Sharding: batch (2) x head-group (4 heads = 256 contiguous features) -> 8
cores. Each core computes q/k/v projections for its 256 output features
from its batch's full activations, then causal attention for its 4 heads.
No collectives: the host assembles the 8 shards.

v7 design (from the 148-178us v4 baseline; all deltas trace-measured):
  - Row-tiled score pairs: heads (2h, 2h+1) live in partition halves of
    qT/kT, so their K=64 score matmuls run CONCURRENTLY on PE row-groups
    (0,0)/(64,0) via tile_position auto-derive (observed dstart ~2-4ns) —
    scores ~29us -> ~15us of PE time.
  - Batched exp: per kc, both heads' scores land in one 2-bank
    [128, 2(head), 512] PSUM tile (pool bufs=2) and ONE q0-trimmed
    activation covers them: 80 calls at the measured pace
    (172 + FD)/1.2GHz instead of 160 smaller ones. The pair tiles
    double-buffer so the next unit's scores only WAR-wait on the
    matching exp (a single 4-bank quad serialized the PE and let HAM
    re-throttle the clock: 219us!). u-pool bufs MUST stay >= 6: bufs=4
    reproducibly corrupts results (rel err 2.6e-2).
  - Causal mask applied AFTER exp as a 0/1 bf16 lower-triangle multiply
    on the DVE ([128,128] per diag block). GPSIMD is useless for this
    (TensorTensor there measured 1151ns vs 260ns on DVE, and it cannot
    access PSUM at all).
  - NO on-device softmax normalization or z transpose: PV's full-width
    [64 v | ones | 63 junk] stationary yields unnormalized z plus the
    sums row in PSUM rows 0:65; those are cast bf16 and DMA'd out as
    [65, 512] blocks; the HOST divides and transposes (also slightly
    more accurate: 5.3e-3 vs 5.6e-3).
  - v bias via a host-replicated [128, 256] tile folded into the v_aug
    fill (tensor_add), replacing 32 K=1 bias matmuls; q/k bias adds for
    the first two s-groups run on the (then idle) Act engine.
  - PE warm-up: 36 dummy matmuls on ones during the input-DMA wait so
    HAM un-throttles (4/8 -> 8/8 clock) before real work.
  - DMA: issues cost ~0.65us each on the issuing queue and all queues
    share ONE 16-engine ring (~310GB/s): the first wave (wq, x-sg0, wk)
    is ordered per-queue FIFO ahead of the 3MB of later s-groups so the
    critical set wins the ring. Outputs ship per query-group (one 3D-AP
    issue each), the final group split by head-pair.
  - NOTE: DMAs sourced from f32r-declared DRAM tensors corrupt DGE
    descriptors (found empirically); DRAM tensors are f32/bf16 only.
  - Budget at ~132us: preamble+ramp ~11, tensor-window ~112 busy /
    ~7 gaps, teardown tail ~12 (framework sem-reset storm + barrier).
"""

import sys

import ml_dtypes
import numpy as np

sys.path.insert(0, "/opt/trn_rl_repo")

import concourse.bass as bass
import concourse.tile as tile
from concourse import bacc, mybir
from concourse.bass_utils import run_bass_kernel_spmd

B, S, D, H = 2, 2048, 1024, 16
DK = D // H  # 64
NCORES = 8
HD = 256  # output features per core (4 heads x 64)
NHC = 4  # heads per core
NST = S // 128  # 16 s-tiles
NCC = D // 128  # 8 contraction chunks
NG = S // 512  # 4 query groups of 512
VW = 128  # v_aug stationary width: 64 v + 1 ones + 63 zeros (full PE width)
ZR = DK + 1  # z output rows per head: 64 z + 1 softmax-sums (host divides)

f32 = mybir.dt.float32
f32r = mybir.dt.float32r
bf16 = mybir.dt.bfloat16
AF = mybir.ActivationFunctionType
PSUM = bass.MemorySpace.PSUM


def _body(nc, tc, xt, wqt, wkt, wvt, consts, maskt, bv, out):
    with (
        tc.tile_pool(name="persist", bufs=1) as persist,
        tc.tile_pool(name="u", bufs=6) as u_pool,
        tc.tile_pool(name="zsb", bufs=2) as zsb_pool,
        tc.tile_pool(name="psum_sc", bufs=2, space=PSUM) as psum_sc,
        tc.tile_pool(name="psum_pr", bufs=2, space=PSUM) as psum_pr,
        tc.tile_pool(name="psum_z", bufs=2, space=PSUM) as psum_z,
    ):
        # ---- persistent operand tensors (all bf16) ----
        xT = persist.tile([128, NCC, S], bf16)  # 32KB/partition
        wqT = persist.tile([128, NCC, HD], bf16)
        wkT = persist.tile([128, NCC, HD], bf16)
        wvT = persist.tile([128, NCC, HD], bf16)
        # qT/kT: head-pair hdc in [128, hdc, S]; head 2*hdc in rows 0:64,
        # head 2*hdc+1 in rows 64:128 (feeds PE row-tiles directly).
        qT = persist.tile([128, 2, S], bf16)
        kT = persist.tile([128, 2, S], bf16)
        v_aug = persist.tile([128, NST, NHC, VW], bf16)
        # lower-triangle 0/1 mask (c >= p), duplicated in both hh slots so
        # one tensor_mul masks both heads' diag blocks.
        mski = persist.tile([128, 2, 128], bf16)
        cst = persist.tile([128, 4], f32)  # bq (2 cols), bk (2 cols)
        bvb = persist.tile([128, HD], bf16)  # b_v replicated across partitions

        # ---- input DMAs: 4 parallel HWDGE queues; ~0.62us issue cost
        # each, transfers ~0.3-3us via 16-engine rings. First wave feeds
        # gen_qk(0): wq on sync, x sg0 (split cc0-1 / cc2-4 / cc5-7) on
        # scalar, wk on vector, wv+consts on gpsimd.
        def pcc(ap, ncc):  # DRAM [ncc*128, w] -> iterate partition-outermost
            return ap.rearrange("(cc p) c -> p cc c", cc=ncc)

        # The critical first wave (wq | x-sg0 | wk, one queue each) must win
        # the SHARED 16-engine DMA ring; everything else queues BEHIND it on
        # the per-queue FIFOs so it can't steal ring bandwidth at startup.
        # Weights arrive as COLUMN halves matching the hdc-serial projection
        # chains' consumption order: wq_h0, wq_h1, wk_h0, wk_h1.
        nc.sync.dma_start(out=wqT[:, :, 0:128], in_=pcc(wqt[:, 0:128], NCC))
        nc.scalar.dma_start(out=xT[:, 0:2, 0:512], in_=pcc(xt[0:256, 0:512], 2))
        nc.gpsimd.dma_start(out=xT[:, 5:8, 0:512], in_=pcc(xt[640:1024, 0:512], 3))
        nc.sync.dma_start(out=wqT[:, :, 128:256], in_=pcc(wqt[:, 128:256], NCC))
        nc.scalar.dma_start(out=xT[:, 2:5, 0:512], in_=pcc(xt[256:640, 0:512], 3))
        nc.gpsimd.dma_start(out=cst[:], in_=consts)
        nc.sync.dma_start(out=wkT[:, :, 0:128], in_=pcc(wkt[:, 0:128], NCC))
        nc.gpsimd.dma_start(out=bvb[:], in_=bv[:])
        nc.gpsimd.dma_start(out=mski[:], in_=maskt)
        nc.sync.dma_start(out=wkT[:, :, 128:256], in_=pcc(wkt[:, 128:256], NCC))
        nc.scalar.dma_start(out=wvT[:], in_=pcc(wvt[:], NCC))
        nc.sync.dma_start(out=xT[:, :, 512:1024], in_=pcc(xt[:, 512:1024], NCC))
        nc.gpsimd.dma_start(out=xT[:, :, 1536:2048], in_=pcc(xt[:, 1536:2048], NCC))
        nc.sync.dma_start(out=xT[:, :, 1024:1536], in_=pcc(xt[:, 1024:1536], NCC))

        ones_row = persist.tile([1, 128], bf16)
        nc.vector.memset(ones_row[:], 1.0)
        # PE warm-up: HAM un-throttles after ~3.4us of sustained matmul
        # activity; burn the DMA wait so real matmuls start at 2.4 GHz.
        warm = psum_pr.tile([128, 128], f32, tag="pr", name="warm")
        for _ in range(36):
            nc.tensor.matmul(
                warm[:], lhsT=ones_row[:], rhs=ones_row[:], start=True, stop=True
            )
        # v_aug: zero everything once (contiguous, 4x DVE mode), then the
        # ones column; v copies later fill cols 0:64 per (st, h).
        nc.vector.memset(v_aug[:], 0.0)
        nc.vector.memset(v_aug[:, :, :, 64], 1.0)

        # ---- projections for s-group sg (yields ~0.9us sub-units) ----
        def gen_qk(sg):
            # q/k: out [hd(128) x 512] per hdc bank, accumulate over 8 ccs.
            # hdc chains run back-to-back (not interleaved) so each chain
            # only depends on its own 0.25MB weight column-half — the DMAs
            # deliver halves in consumption order during the ramp.
            for wT_t, bcol, dstT in ((wqT, 0, qT), (wkT, 2, kT)):
                pa = psum_pr.tile([128, 512], f32, tag="pr", name="pa")
                pb = psum_pr.tile([128, 512], f32, tag="pr", name="pb")
                for hdc, pp in ((0, pa), (1, pb)):
                    for cc in range(NCC):
                        nc.tensor.matmul(
                            pp[:],
                            lhsT=wT_t[:, cc, bass.ts(hdc, 128)],
                            rhs=xT[:, cc, bass.ts(sg, 512)],
                            start=(cc == 0),
                            stop=(cc == NCC - 1),
                        )
                        if cc % 4 == 3:
                            if cc == NCC - 1:
                                if sg < 2:
                                    # ramp: scalar is idle before first exps
                                    nc.scalar.add(
                                        dstT[:, hdc, bass.ts(sg, 512)],
                                        pp[:],
                                        cst[:, bcol + hdc : bcol + hdc + 1],
                                    )
                                else:
                                    nc.vector.tensor_scalar_add(
                                        dstT[:, hdc, bass.ts(sg, 512)],
                                        pp[:],
                                        cst[:, bcol + hdc : bcol + hdc + 1],
                                    )
                            yield

        def gen_v(sg, spairs=(0, 1)):
            # v: natural [s(128) x 256] per s-tile, pairs alternate banks
            for spair in spairs:
                pvs = [
                    psum_pr.tile([128, HD], f32, tag="pr", name=f"pv{i}")
                    for i in range(2)
                ]
                for cb in range(2):
                    for cc in range(4 * cb, 4 * cb + 4):
                        for stl in range(2):
                            nc.tensor.matmul(
                                pvs[stl][:],
                                lhsT=xT[:, cc, bass.ts(sg * 4 + spair * 2 + stl, 128)],
                                rhs=wvT[:, cc, :],
                                start=(cc == 0),
                                stop=(cc == NCC - 1),
                            )
                    if cb == 1:
                        for stl in range(2):
                            st = sg * 4 + spair * 2 + stl
                            nc.vector.tensor_add(
                                v_aug[:, st, :, 0:64],
                                pvs[stl][:].rearrange("p (h d) -> p h d", h=NHC),
                                bvb[:].rearrange("p (h d) -> p h d", h=NHC),
                            )
                    yield

        # ---- attention for query group g (512 queries) ----
        def gen_attn(g):
            nkc = 4 * g + 4
            # per-group output staging: unnormalized z (64 rows) + sums row
            # per head; host divides and transposes.
            zsb = zsb_pool.tile([ZR, NHC, 512], bf16, tag="zs", name="zsb")
            for hdc in (0, 1):
                zps = [
                    psum_z.tile([VW, 512], f32, tag="z", name=f"zp{hh}")
                    for hh in range(2)
                ]
                prev = None  # (kb, u_j0, u_j1)

                def flush_pv(kb, u0, u1):
                    # PV for the 4 (head, kc) units of unit kb; trim q0
                    for j, u in ((0, u0), (1, u1)):
                        kc = kb + j
                        q0 = max(0, 128 * (kc - 4 * g))
                        for hh in (0, 1):
                            nc.tensor.matmul(
                                zps[hh][:, q0:512],
                                lhsT=v_aug[:, kc, 2 * hdc + hh, :],
                                rhs=u[:, hh, q0:512],
                                start=(kc == 0),
                                stop=(kc == nkc - 1),
                            )

                for kb in range(0, nkc, 2):
                    # scores: per kc a 2-bank [128, 2(head), 512] tile; the
                    # two heads' matmuls run concurrently on PE row-groups
                    # (0,0)/(64,0). One exp per kc covers both heads
                    # (q0-trimmed strided AP). bufs=2 double-buffers so the
                    # next unit's scores only wait on the matching exp.
                    us = []
                    for j in (0, 1):
                        kc = kb + j
                        q0 = max(0, 128 * (kc - 4 * g))
                        sp = psum_sc.tile([128, 2, 512], f32, tag="sc", name="sp")
                        for hh, po in ((0, 0), (1, 64)):
                            nc.tensor.matmul(
                                sp[:, hh, q0:512],
                                lhsT=kT[po : po + 64, hdc, bass.ts(kc, 128)],
                                rhs=qT[po : po + 64, hdc, bass.ds(g * 512 + q0, 512 - q0)],
                                start=True,
                                stop=True,
                            )
                        u = u_pool.tile([128, 2, 512], bf16, tag="u", name="u")
                        nc.scalar.activation(
                            u[:, :, q0:512], sp[:, :, q0:512], AF.Exp, scale=0.125
                        )
                        # causal masking: zero upper triangle of diag block
                        if q0 > 0 or kc == 4 * g:
                            for hh in (0, 1):
                                nc.vector.tensor_mul(
                                    u[:, hh, q0 : q0 + 128],
                                    u[:, hh, q0 : q0 + 128],
                                    mski[:, 0, :],
                                )
                        us.append(u)
                    if prev is not None:
                        flush_pv(*prev)
                    prev = (kb, us[0], us[1])
                    yield
                flush_pv(*prev)

                # z tail: cast [65, 512] (64 z rows + sums row) straight to
                # bf16 SBUF; normalization and transpose happen on the host.
                # (NB: a scalar.copy variant measured 22us slower, but during
                # a suspected P0-downclock episode; AF.Copy may also force an
                # ACT table-set switch away from Exp. Keep casts on the DVE.)
                for hh in (0, 1):
                    nc.vector.tensor_copy(
                        zsb[0:ZR, 2 * hdc + hh, :], zps[hh][0:ZR, :]
                    )
                    yield
                if g == NG - 1:
                    # tail group: ship each head-pair as soon as it's done.
                    # Issues go on gpsimd/sync — NOT scalar, whose exp stream
                    # paces the attention phases (an issue costs ~0.9us).
                    eng = (nc.gpsimd, nc.sync)[hdc]
                    eng.dma_start(
                        out=out[
                            bass.ds(hdc * 2 * ZR, 2 * ZR), bass.ts(g, 512)
                        ].rearrange("(h d) c -> d h c", h=2),
                        in_=zsb[0:ZR, 2 * hdc : 2 * hdc + 2, :],
                    )
            if g < NG - 1:
                # one issue per group (rows h*ZR+d of out) on idle gpsimd
                nc.gpsimd.dma_start(
                    out=out[:, bass.ts(g, 512)].rearrange("(h d) c -> d h c", h=NHC),
                    in_=zsb[0:ZR, :, :],
                )
            yield

        def drain(gen):
            for _ in gen:
                pass

        # program-order interleave: attention for group g alternates with the
        # projection sub-units of s-group g+1 so every engine queue mixes both
        # work streams.
        def chain2(a, b):
            yield from a
            yield from b

        drain(gen_qk(0))
        drain(gen_v(0, (0,)))
        for sg in range(NG):
            a = gen_attn(sg)
            # v projections carry no Act-engine work, so fractions of them are
            # deferred into the Act-bound later phases: half of v(0) leads the
            # attn(0) filler (so the first scores aren't queued behind all of
            # v(0) on the PE), half of v(2) into attn(2), v(3) into attn(3).
            if sg == 0:
                f = chain2(gen_v(0, (1,)), chain2(gen_qk(1), gen_v(1)))
            elif sg == 1:
                f = chain2(gen_qk(2), gen_v(2, (0,)))
            elif sg == 2:
                f = chain2(gen_v(2, (1,)), gen_qk(3))
            else:
                f = gen_v(3)
            rate = 2 if sg == NG - 1 else 1  # spread v(NG-1) across the tail
            rnd = 0
            while True:
                sa = next(a, StopIteration)
                sf = next(f, StopIteration) if rnd % rate == rate - 1 else None
                rnd += 1
                if sa is StopIteration:
                    drain(f)
                    break
                del sf


def build():
    nc = bacc.Bacc(
        "TRN2", target_bir_lowering=False, debug=False, num_devices=NCORES
    )
    xt = nc.dram_tensor("xt", [D, S], bf16, kind="ExternalInput")
    wqt = nc.dram_tensor("wqt", [D, HD], bf16, kind="ExternalInput")
    wkt = nc.dram_tensor("wkt", [D, HD], bf16, kind="ExternalInput")
    wvt = nc.dram_tensor("wvt", [D, HD], bf16, kind="ExternalInput")
    consts = nc.dram_tensor("consts", [128, 4], f32, kind="ExternalInput")
    maskt = nc.dram_tensor("maskt", [128, 2 * 128], bf16, kind="ExternalInput")
    bv = nc.dram_tensor("bv", [128, HD], bf16, kind="ExternalInput")
    out = nc.dram_tensor("out", [NHC * ZR, S], bf16, kind="ExternalOutput")
    with tile.TileContext(nc) as tc:
        _body(
            nc, tc, xt.ap(), wqt.ap(), wkt.ap(), wvt.ap(),
            consts.ap(), maskt.ap(), bv.ap(), out.ap(),
        )
    nc.compile()
    return nc


_NC_CACHE = None


def _get_nc():
    global _NC_CACHE
    if _NC_CACHE is None:
        _NC_CACHE = build()
    return _NC_CACHE


def make_in_maps(q_input, W_q, b_q, W_k, b_k, W_v, b_v):
    bf = ml_dtypes.bfloat16
    # masks+ident packed: [:, 0:128] lower-triangle (c >= p), [:, 128:256] identity
    ii = np.arange(128)
    maskt = np.zeros((128, 2 * 128), np.float32)
    maskt[:, 0:128] = (ii[None, :] >= ii[:, None]).astype(np.float32)
    maskt[:, 128:256] = maskt[:, 0:128]
    maskt = maskt.astype(bf)
    # host-side marshaling: bf16 cast + transpose (kernel-internal layout)
    xts = [np.ascontiguousarray(np.asarray(q_input[b]).T.astype(bf)) for b in range(B)]
    in_maps = []
    for c in range(NCORES):
        b = c // 4
        hs = slice((c % 4) * HD, (c % 4 + 1) * HD)
        consts = np.zeros((128, 4), np.float32)
        consts[:, 0:2] = np.asarray(b_q[hs], dtype=np.float32).reshape(2, 128).T
        consts[:, 2:4] = np.asarray(b_k[hs], dtype=np.float32).reshape(2, 128).T
        in_maps.append(
            {
                "xt": xts[b],
                "wqt": np.ascontiguousarray(np.asarray(W_q[hs]).T.astype(bf)),
                "wkt": np.ascontiguousarray(np.asarray(W_k[hs]).T.astype(bf)),
                "wvt": np.ascontiguousarray(np.asarray(W_v[hs]).T.astype(bf)),
                "consts": consts,
                "maskt": maskt,
                "bv": np.ascontiguousarray(
                    np.broadcast_to(
                        np.asarray(b_v[hs]).astype(bf).reshape(1, HD), (128, HD)
                    )
                ),
            }
        )
    return in_maps


def assemble(results):
    full = np.empty((B, S, D), dtype=np.float32)
    for c in range(NCORES):
        b = c // 4
        raw = np.asarray(results[c]["out"]).astype(np.float32)  # [NHC*ZR, S]
        zhds = raw.reshape(NHC, ZR, S)  # per head: 64 z rows + sums row
        zn = zhds[:, 0:DK, :] / zhds[:, DK : DK + 1, :]  # normalize
        h0 = (c % 4) * HD
        for h in range(NHC):
            full[b, :, h0 + h * DK : h0 + (h + 1) * DK] = zn[h].T
    return full


def _ensure_ntff_hook():
    """Register the axon NTFF profiling hook if the image's antenv lacks it."""
    try:
        from antenv import axon_hooks  # noqa: F401

        return
    except ImportError:
        pass
    import types

    try:
        from trn_agent_boot.trn_boot import _ntff_profile_via_ctypes

        hook = _ntff_profile_via_ctypes("/opt/axon/libaxon_pjrt.so")
    except Exception:
        hook = None
    mod = types.ModuleType("antenv.axon_hooks")
    mod._hook = hook
    mod.get_axon_ntff_profile_hook = lambda: mod._hook

    def _set(h):
        mod._hook = h

    mod.set_axon_ntff_profile_hook = _set
    sys.modules["antenv.axon_hooks"] = mod
    try:
        import antenv

        antenv.axon_hooks = mod
    except ImportError:
        pass


def run(inputs_dict, trace=False):
    """Run on hardware; returns (full_output, BassKernelResults)."""
    nc = _get_nc()
    if trace:
        _ensure_ntff_hook()
        import concourse.bass_utils as _bu

        _bu.upload_artifacts = lambda d: d  # no bucket access in this env
    in_maps = make_in_maps(**{k: np.asarray(v) for k, v in inputs_dict.items()})
    res = run_bass_kernel_spmd(nc, in_maps, core_ids=list(range(NCORES)), trace=trace)
    return assemble(res.results), res


def kernel(**inputs):
    out, _ = run(inputs, trace=False)
    return out
